# revision 1
# baseline (speedup 1.0000x reference)
"""Trainium2 Bass kernel for nn_FDSM_40295383171690.

Math (verified vs reference in fp64, rel err ~4e-7):
  gating: GN(concat(x,x)) == concat(GN4(x), GN4(x)); fold gamma/beta into the
          1x1 conv -> W', b'; weights = softmax(wg @ GAP(relu(W' xn + b')))
  fft:    out = irfft2( rfft2(x)^2 * Wmix ) + r*x
          Wmix[b] = sum_f weights[b,f] * Wsym[f],  Wsym = ds_w filters with
          columns k2 in {0,64} Hermitian-symmetrized along k1 (this absorbs
          the rfft2(irfft2(.)) Hermitian projection exactly).

Sharding: core k = gating for sample k (all C) + FFT branch for channels
[8k,8k+8) of all samples; the [8,4] gating weights are AllGathered on-chip.

DFTs are dense matmuls: stage1 (contract h, lhsT=x, rhs=[Ch|Sh], fp32r N=256),
stage2 (contract w, lhsT=U/V bf16, rhs=[Cw|-Sw],[-Sw|-Cw]), iDFT-A (contract
k1, lhsT=D fp32r, rhs=[Cih|Sih],[-Sih|Cih] N=256), iDFT-B (contract k2,
lhsT=Z2 bf16, rhs=Gc,Gs). Wmix is a K=(f x 32)-packed matmul with col-tiling.
"""

import numpy as np
import ml_dtypes

import concourse.bass as bass
import concourse.bacc as bacc
import concourse.mybir as mybir
import concourse.tile as tile
from concourse.bass_utils import run_bass_kernel_spmd

dt = mybir.dt
AF = mybir.ActivationFunctionType
ALU = mybir.AluOpType
AX = mybir.AxisListType

B, C, H, W, F = 8, 64, 128, 128, 4
WF = 65
NCORES = 8
CS = C // NCORES
EPS = 1e-5
HW = H * W

_cache = {}
DEBUG = False
N_B = 8
SIM_MODE = False


def _build_constants():
    h = np.arange(H)
    k1 = np.arange(H)
    w = np.arange(W)
    k2 = np.arange(WF)
    Ch = np.cos(2 * np.pi * np.outer(h, k1) / H).astype(np.float32)
    Sh = np.sin(2 * np.pi * np.outer(h, k1) / H).astype(np.float32)
    Cw = np.cos(2 * np.pi * np.outer(w, k2) / W).astype(np.float32)
    Sw = np.sin(2 * np.pi * np.outer(w, k2) / W).astype(np.float32)
    Cih = (np.cos(2 * np.pi * np.outer(k1, h) / H) / H).astype(np.float32)
    Sih = (np.sin(2 * np.pi * np.outer(k1, h) / H) / H).astype(np.float32)
    cj = np.ones(WF, np.float32)
    cj[1:64] = 2.0
    Gc = (cj[:, None] * np.cos(2 * np.pi * np.outer(k2, w) / W) / W).astype(np.float32)
    Gs = (-cj[:, None] * np.sin(2 * np.pi * np.outer(k2, w) / W) / W).astype(np.float32)

    bf = ml_dtypes.bfloat16
    consts = {
        "R1": np.concatenate([Ch, Sh], 1),
        "R2a": np.concatenate([Cw, -Sw], 1),
        "R2b": np.concatenate([-Sw, -Cw], 1),
        "RA1": np.concatenate([Cih, Sih], 1),
        "RA2": np.concatenate([-Sih, Cih], 1),
        "RB1": Gc,
        "RB2": Gs,
    }
    G16 = np.zeros((128, 16), np.float32)
    E16 = np.zeros((16, 128), np.float32)
    for p in range(128):
        g = (p % 64) // 4
        G16[p, g] = 1.0
        E16[g, p] = 1.0
    F2 = np.zeros((128, 64), np.float32)
    for p in range(128):
        F2[p, p % 64] = 1.0 / HW
    E4 = np.zeros((4, 128), np.float32)
    for p in range(128):
        E4[p // 32, p] = 1.0
    maskJ = np.zeros((4, 128, 128), np.float32)
    for J in range(4):
        for p in range(128):
            f, pp = p // 32, p % 32
            maskJ[J, p, 32 * J + pp] = 1.0
    consts.update({"G16": G16, "E16": E16, "F2": F2, "E4": E4,
                   "maskJ": maskJ})
    return consts


def _prep_params(inputs):
    gamma = np.asarray(inputs["gn_gamma"], np.float64)
    beta = np.asarray(inputs["gn_beta"], np.float64)
    agg_w = np.asarray(inputs["agg_w"], np.float64)
    agg_b = np.asarray(inputs["agg_b"], np.float64)
    wg_w = np.asarray(inputs["wg_w"], np.float64)
    wg_b = np.asarray(inputs["wg_b"], np.float64)

    Wp = agg_w[:, :C] * gamma[None, :C] + agg_w[:, C:] * gamma[None, C:]
    bp = agg_w[:, :C] @ beta[:C] + agg_w[:, C:] @ beta[C:] + agg_b
    Wblk = np.zeros((128, 128), np.float32)
    for t in range(2):
        Wblk[64 * t:64 * t + 64, 64 * t:64 * t + 64] = Wp.T.astype(np.float32)
    bprime = np.zeros((128, 1), np.float32)
    bprime[:64, 0] = bp.astype(np.float32)
    bprime[64:, 0] = bp.astype(np.float32)
    WgT = wg_w.T.astype(np.float32)
    wgb = wg_b.astype(np.float32).reshape(1, 4)

    ds = np.asarray(inputs["ds_w"], np.float64)
    Wc = ds[..., 0] + 1j * ds[..., 1]                     # [F,C,H(k1),WF(k2)]
    rev = (-np.arange(H)) % H
    Wt = Wc.copy()
    for j in (0, WF - 1):
        Wt[..., j] = 0.5 * (Wc[..., j] + np.conj(Wc[:, :, rev, j]))
    rw = float(np.asarray(inputs["residual_weight"]).ravel()[0])
    return Wblk, bprime, WgT, wgb, Wt, rw


def _build_kernel():
    bf16, f32, f32r = dt.bfloat16, dt.float32, dt.float32r

    nc = bacc.Bacc("TRN2", target_bir_lowering=False, debug=False,
                   num_devices=NCORES)

    d = {}
    d["featf"] = nc.dram_tensor("featf", [128, B * CS * W], f32r,
                                kind="ExternalInput").ap()
    d["featg"] = nc.dram_tensor("featg", [128, 64 * 128], f32,
                                kind="ExternalInput").ap()
    d["ftiles"] = nc.dram_tensor("ftiles", [4, 128, CS * 2 * WF], f32r,
                                 kind="ExternalInput").ap()
    d["maskJ"] = nc.dram_tensor("maskJ", [4, 128, 128], f32,
                                kind="ExternalInput").ap()
    for name, shape, dty in [
        ("R1", [128, 256], f32r), ("R2a", [128, 130], f32),
        ("R2b", [128, 130], f32), ("RA1", [128, 256], f32r),
        ("RA2", [128, 256], f32r), ("RB1", [65, 128], f32),
        ("RB2", [65, 128], f32), ("G16", [128, 16], f32),
        ("E16", [16, 128], f32), ("F2", [128, 64], f32),
        ("E4", [4, 128], f32),
        ("Wblk", [128, 128], f32), ("bprime", [128, 1], f32),
        ("WgT", [64, 4], f32), ("wgb", [1, 4], f32),
        ("rcol", [128, 1], f32),
    ]:
        d[name] = nc.dram_tensor(name, shape, dty, kind="ExternalInput").ap()
    out_d = nc.dram_tensor("out", [B, CS, H, W], f32, kind="ExternalOutput").ap()
    if DEBUG:
        dbg = {
            "d_stats": nc.dram_tensor("d_stats", [128, 2], f32, kind="ExternalOutput").ap(),
            "d_gs": nc.dram_tensor("d_gs", [16, 6], f32, kind="ExternalOutput").ap(),
            "d_nstat": nc.dram_tensor("d_nstat", [128, 2], f32, kind="ExternalOutput").ap(),
            "d_gap": nc.dram_tensor("d_gap", [128, 16], f32, kind="ExternalOutput").ap(),
            "d_pooled": nc.dram_tensor("d_pooled", [64, 1], f32, kind="ExternalOutput").ap(),
            "d_logit": nc.dram_tensor("d_logit", [1, 8], f32, kind="ExternalOutput").ap(),
            "d_wrow": nc.dram_tensor("d_wrow", [1, 4], f32, kind="ExternalOutput").ap(),
            "d_wcol": nc.dram_tensor("d_wcol", [128, 8], f32, kind="ExternalOutput").ap(),
            "d_xn": nc.dram_tensor("d_xn", [128, 512], f32, kind="ExternalOutput").ap(),
            "d_y": nc.dram_tensor("d_y", [128, 512], f32, kind="ExternalOutput").ap(),
            "d_wb": nc.dram_tensor("d_wb", [128, 128], f32, kind="ExternalOutput").ap(),
            "d_wb2": nc.dram_tensor("d_wb2", [128, 128], f32, kind="ExternalOutput").ap(),
        }

    with tile.TileContext(nc) as tc:
        with (
            tc.tile_pool(name="consts", bufs=1) as cp,
            tc.tile_pool(name="feat", bufs=1) as fp,
            tc.tile_pool(name="gate", bufs=1) as gp,
            tc.tile_pool(name="work", bufs=3) as wkp,
            tc.tile_pool(name="sgrp", bufs=2) as sgp,
            tc.tile_pool(name="outp", bufs=3) as op_,
            tc.tile_pool(name="ps_a", bufs=2, space="PSUM") as ps_a,
            tc.tile_pool(name="ps_b", bufs=2, space="PSUM") as ps_b,
            tc.tile_pool(name="ps_c", bufs=2, space="PSUM") as ps_c,
            tc.tile_pool(name="ps_d", bufs=1, space="PSUM") as ps_d,
            tc.tile_pool(name="ps_m", bufs=1, space="PSUM") as ps_m,
            tc.tile_pool(name="dram", bufs=1, space="DRAM") as dr,
        ):
            ct = {}
            for name in ["R1", "R2a", "R2b", "RA1", "RA2", "RB1", "RB2",
                         "G16", "E16", "F2", "E4", "Wblk",
                         "bprime", "WgT", "wgb", "rcol"]:
                t = cp.tile(list(d[name].shape), d[name].dtype, tag=name)
                nc.sync.dma_start(t[:], d[name][:])
                ct[name] = t
            for name in ["R2a", "R2b", "RB1", "RB2"]:
                t = cp.tile(list(d[name].shape), bf16, tag=name + "b")
                nc.vector.tensor_copy(t[:], ct[name][:])
                ct[name] = t

            maskt = []
            for J in range(4):
                t = cp.tile([128, 128], f32, tag=f"maskJ{J}")
                nc.sync.dma_start(t[:], d["maskJ"][J])
                maskt.append(t)

            featb = []
            for b in range(B):
                t = fp.tile([128, CS * W], f32r, tag=f"featb{b}")
                nc.sync.dma_start(t[:], d["featf"][:, b * CS * W:(b + 1) * CS * W])
                featb.append(t)
            featg = fp.tile([128, 64 * 128], f32, tag="featg")
            nc.sync.dma_start(featg[:], d["featg"][:])
            ftl = []
            for J in range(4):
                t = fp.tile([128, CS * 2 * WF], f32r, tag=f"ftl{J}")
                nc.sync.dma_start(t[:], d["ftiles"][J])
                ftl.append(t)

            # ================= gating (sample b = core id) ===================
            xn = gp.tile([128, 64 * 128], f32r, tag="xn")
            wblkr = gp.tile([128, 128], f32r, tag="wblkr")
            nc.vector.tensor_copy(wblkr[:], ct["Wblk"][:])
            stats = gp.tile([128, 2], f32, tag="stats")
            nc.vector.tensor_scalar(xn[:], featg[:], 1.0, 0.0, ALU.mult,
                                    ALU.add, accum_out=stats[:, 0:1])
            nc.scalar.activation(xn[:], featg[:], AF.Square,
                                 accum_out=stats[:, 1:2])
            gstat = ps_m.tile([16, 2], f32, tag="pmix")
            nc.tensor.matmul(gstat[:], ct["G16"][:], stats[:])
            gs = gp.tile([16, 6], f32, tag="gs")
            nc.scalar.mul(gs[:, 0:1], gstat[:, 0:1], 1.0 / (4 * HW))
            nc.scalar.mul(gs[:, 1:2], gstat[:, 1:2], 1.0 / (4 * HW))
            nc.scalar.activation(gs[:, 2:3], gs[:, 0:1], AF.Square)
            nc.vector.tensor_sub(gs[:, 3:4], gs[:, 1:2], gs[:, 2:3])
            epst = gp.tile([16, 1], f32, tag="epst")
            nc.vector.memset(epst[:], EPS)
            nc.scalar.activation(gs[:, 4:5], gs[:, 3:4], AF.Sqrt,
                                 bias=epst[:, 0:1])
            nc.vector.reciprocal(gs[:, 5:6], gs[:, 4:5])
            gs2 = gp.tile([16, 2], f32, tag="gs2")
            nc.vector.tensor_mul(gs2[:, 0:1], gs[:, 0:1], gs[:, 5:6])
            nc.vector.tensor_scalar_mul(gs2[:, 0:1], gs2[:, 0:1], -1.0)
            nc.vector.tensor_copy(gs2[:, 1:2], gs[:, 5:6])
            pstat = ps_m.tile([128, 2], f32, tag="pmix")
            nc.tensor.matmul(pstat[:], ct["E16"][:], gs2[:])
            nstat = gp.tile([128, 2], f32, tag="nstat")
            nc.scalar.copy(nstat[:], pstat[:])
            nc.scalar.activation(xn[:], featg[:], AF.Identity,
                                 bias=nstat[:, 0:1], scale=nstat[:, 1:2])
            if DEBUG:
                wbf = gp.tile([128, 128], f32, tag="wbf")
                nc.vector.tensor_copy(wbf[:], wblkr[:])
                nc.sync.dma_start(dbg["d_wb"][:], wbf[:])
                xnf = gp.tile([128, 512], f32, tag="xnf")
                nc.vector.tensor_copy(xnf[:], xn[:, 0:512])
                nc.sync.dma_start(dbg["d_xn"][:], xnf[:])
            gap = gp.tile([128, 16], f32, tag="gap")
            for j in range(16):
                yp = ps_a.tile([128, 512], f32, tag="p1")
                nc.tensor.matmul(yp[:], wblkr[:],
                                 xn[:, j * 512:(j + 1) * 512])
                nc.scalar.activation(
                    featg[:, j * 512:(j + 1) * 512], yp[:], AF.Relu,
                    bias=ct["bprime"][:, 0:1], scale=1.0,
                    accum_out=gap[:, j:j + 1])
                if DEBUG and j == 0:
                    yf = gp.tile([128, 512], f32, tag="yf")
                    nc.scalar.copy(yf[:], yp[:])
                    nc.sync.dma_start(dbg["d_y"][:], yf[:])
            gsum = gp.tile([128, 1], f32, tag="gsum")
            nc.vector.tensor_reduce(gsum[:], gap[:], AX.X, ALU.add)
            ppool = ps_m.tile([64, 1], f32, tag="pmix")
            nc.tensor.matmul(ppool[:], ct["F2"][:], gsum[:])
            pooled = gp.tile([64, 1], f32, tag="pooled")
            nc.scalar.copy(pooled[:], ppool[:])
            plog = ps_m.tile([1, 4], f32, tag="pmix")
            nc.tensor.matmul(plog[:], pooled[:], ct["WgT"][:])
            logit = gp.tile([1, 8], f32, tag="logit")
            nc.vector.memset(logit[:], 0.0)
            nc.vector.tensor_add(logit[:, 0:4], plog[:], ct["wgb"][:])
            nc.vector.tensor_reduce(logit[:, 4:5], logit[:, 0:4], AX.X, ALU.max)
            nc.vector.tensor_scalar(logit[:, 0:4], logit[:, 0:4],
                                    logit[:, 4:5], None, ALU.subtract)
            wrow = gp.tile([1, 4], f32, tag="wrow")
            nc.scalar.activation(wrow[:], logit[:, 0:4], AF.Exp,
                                 accum_out=logit[:, 5:6])
            nc.vector.reciprocal(logit[:, 6:7], logit[:, 5:6])
            nc.vector.tensor_scalar(wrow[:], wrow[:], logit[:, 6:7], None,
                                    ALU.mult)
            ag_in = dr.tile([1, 4], f32)
            ag_out = dr.tile([8, 4], f32)
            nc.sync.dma_start(ag_in[:], wrow[:])
            if SIM_MODE:
                for _b in range(8):
                    nc.sync.dma_start(ag_out[_b:_b + 1, :], ag_in[:])
            else:
                nc.gpsimd.collective_compute(
                    "AllGather", ALU.bypass, ins=[ag_in.opt()],
                    outs=[ag_out.opt()],
                    replica_groups=[list(range(NCORES))],
                )
            wT = gp.tile([4, 8], f32, tag="wT")
            nc.sync.dma_start(wT[:], ag_out[:].rearrange("b f -> f b"))
            pwcol = ps_m.tile([128, 8], f32, tag="pmix")
            nc.tensor.matmul(pwcol[:], ct["E4"][:], wT[:])
            wcol = gp.tile([128, 8], f32, tag="wcol")
            nc.scalar.copy(wcol[:], pwcol[:])
            if DEBUG:
                nc.sync.dma_start(dbg["d_stats"][:], stats[:])
                nc.sync.dma_start(dbg["d_gs"][:], gs[:])
                nc.sync.dma_start(dbg["d_nstat"][:], nstat[:])
                nc.sync.dma_start(dbg["d_gap"][:], gap[:])
                nc.sync.dma_start(dbg["d_pooled"][:], pooled[:])
                nc.sync.dma_start(dbg["d_logit"][:], logit[:])
                nc.sync.dma_start(dbg["d_wrow"][:], wrow[:])
                nc.sync.dma_start(dbg["d_wcol"][:], wcol[:])
            wpat = []
            for b in range(B):
                row = []  # d_wb2 dump appended after wpat build below
                for J in range(4):
                    t = gp.tile([128, 128], f32r, tag=f"wpat{b}_{J}")
                    nc.scalar.activation(t[:], maskt[J][:], AF.Identity,
                                         scale=wcol[:, b:b + 1])
                    row.append(t)
                wpat.append(row)

            if DEBUG:
                zz = gp.tile([128, 128], bf16, tag="zz")
                nc.vector.tensor_scalar(zz[:], wpat[7][3][:], 0.0, None, ALU.mult)
                wb2 = gp.tile([128, 128], f32, tag="wb2")
                nc.vector.tensor_add(wb2[:], ct["Wblk"][:], zz[:])
                nc.sync.dma_start(dbg["d_wb2"][:], wb2[:])
            # ================= FFT branch ====================================
            for b in range(N_B):
                fb = featb[b]
                for g in range(2):                      # 4-chain groups
                    c0 = 4 * g
                    Sr4 = sgp.tile([128, 260], f32, tag="Sr4")
                    Si4 = sgp.tile([128, 260], f32, tag="Si4")
                    Xi4 = sgp.tile([128, 260], f32, tag="Xi4")
                    Dr4 = sgp.tile([128, 260], f32r, tag="Dr4")
                    Di4 = sgp.tile([128, 260], f32r, tag="Di4")
                    Wm4 = sgp.tile([128, 520], f32, tag="Wm4")
                    m1 = sgp.tile([128, 260], f32, tag="m1")
                    m2 = sgp.tile([128, 260], f32, tag="m2")
                    m3 = sgp.tile([128, 260], f32, tag="m3")
                    m4 = sgp.tile([128, 260], f32, tag="m4")
                    pB = ps_d.tile([128, 512], f32, tag="pB")
                    for cc in range(2):                 # 2-chain psum subgroups
                        ch2 = c0 + 2 * cc
                        pm = ps_m.tile([128, 260], f32, tag="pmix")
                        for J in range(4):
                            nc.tensor.matmul(
                                pm[:], wpat[b][J][:],
                                ftl[J][:, ch2 * 130:(ch2 + 2) * 130],
                                start=(J == 0), stop=(J == 3))
                        p1 = ps_a.tile([128, 512], f32, tag="p1")
                        for j in range(2):
                            c = ch2 + j
                            nc.tensor.matmul(p1[:, j * 256:(j + 1) * 256],
                                             fb[:, c * 128:(c + 1) * 128],
                                             ct["R1"][:])
                        uv = wkp.tile([128, 512], bf16, tag="uv")
                        if cc == 0:
                            nc.vector.tensor_copy(uv[:], p1[:])
                        else:
                            nc.scalar.copy(uv[:], p1[:])
                        p2 = ps_b.tile([128, 260], f32, tag="p2")
                        for j in range(2):
                            nc.tensor.matmul(p2[:, j * 130:(j + 1) * 130],
                                             uv[:, j * 256:j * 256 + 128],
                                             ct["R2a"][:], start=True,
                                             stop=False)
                            nc.tensor.matmul(p2[:, j * 130:(j + 1) * 130],
                                             uv[:, j * 256 + 128:(j + 1) * 256],
                                             ct["R2b"][:], start=False,
                                             stop=True)
                        # strided views: [128, 2chain, 65]
                        p2v = p2[:].rearrange("p (j x) -> p j x", j=2)
                        xr = p2v[:, :, 0:65]
                        xi = p2v[:, :, 65:130]
                        s4 = slice(2 * cc, 2 * cc + 2)
                        srv = Sr4[:].rearrange("p (q x) -> p q x", q=4)[:, s4]
                        siv = Si4[:].rearrange("p (q x) -> p q x", q=4)[:, s4]
                        xiv = Xi4[:].rearrange("p (q x) -> p q x", q=4)[:, s4]
                        m1v = m1[:].rearrange("p (q x) -> p q x", q=4)[:, s4]
                        nc.scalar.activation(srv, xr, AF.Square)   # Xr^2
                        nc.vector.tensor_copy(xiv, xi)             # Xi
                        nc.scalar.activation(m1v, xi, AF.Square)   # Xi^2 (scratch)
                        # Si = 2*Xr*Xi  (one psum operand)
                        nc.vector.scalar_tensor_tensor(siv, xr, 2.0, xiv,
                                                       ALU.mult, ALU.mult)
                        # Sr = Xr^2 - Xi^2
                        nc.vector.tensor_sub(srv, srv, m1v)
                        nc.scalar.copy(Wm4[:, cc * 260:(cc + 1) * 260], pm[:])
                    # ---- D = S * Wmix  (4 chains batched) ----
                    wmv = Wm4[:].rearrange("p (q x) -> p q x", q=4)
                    wmr = wmv[:, :, 0:65]
                    wmi = wmv[:, :, 65:130]
                    sr_f = Sr4[:].rearrange("p (q x) -> p q x", q=4)
                    si_f = Si4[:].rearrange("p (q x) -> p q x", q=4)
                    m1f = m1[:].rearrange("p (q x) -> p q x", q=4)
                    m2f = m2[:].rearrange("p (q x) -> p q x", q=4)
                    m3f = m3[:].rearrange("p (q x) -> p q x", q=4)
                    m4f = m4[:].rearrange("p (q x) -> p q x", q=4)
                    nc.vector.tensor_mul(m1f, sr_f, wmr)
                    nc.vector.tensor_mul(m2f, si_f, wmi)
                    nc.gpsimd.tensor_mul(m3f, sr_f, wmi)
                    nc.gpsimd.tensor_mul(m4f, si_f, wmr)
                    nc.vector.tensor_sub(Dr4[:], m1[:], m2[:])
                    nc.gpsimd.tensor_add(Di4[:], m3[:], m4[:])
                    # ---- iDFT ----
                    for cc in range(2):
                        pA = ps_c.tile([65, 512], f32, tag="pA")
                        for j in range(2):
                            q = 2 * cc + j
                            nc.tensor.matmul(pA[:, j * 256:(j + 1) * 256],
                                             Dr4[:, q * 65:(q + 1) * 65],
                                             ct["RA1"][:], start=True,
                                             stop=False)
                            nc.tensor.matmul(pA[:, j * 256:(j + 1) * 256],
                                             Di4[:, q * 65:(q + 1) * 65],
                                             ct["RA2"][:], start=False,
                                             stop=True)
                        z2 = wkp.tile([65, 512], bf16, tag="z2")
                        if cc == 0:
                            nc.vector.tensor_copy(z2[:], pA[:])
                        else:
                            nc.scalar.copy(z2[:], pA[:])
                        for j in range(2):
                            q = 2 * cc + j
                            nc.tensor.matmul(pB[:, q * 128:(q + 1) * 128],
                                             z2[:, j * 256:j * 256 + 128],
                                             ct["RB1"][:], start=True,
                                             stop=False)
                            nc.tensor.matmul(pB[:, q * 128:(q + 1) * 128],
                                             z2[:, j * 256 + 128:(j + 1) * 256],
                                             ct["RB2"][:], start=False,
                                             stop=True)
                    ot = op_.tile([128, 512], f32, tag="ot")
                    nc.vector.scalar_tensor_tensor(
                        ot[:], fb[:, c0 * 128:(c0 + 4) * 128].bitcast(f32),
                        ct["rcol"][:, 0:1], pB[:], ALU.mult, ALU.add)
                    nc.sync.dma_start(
                        out_d[b, c0:c0 + 4].rearrange("c h w -> h c w"),
                        ot[:].rearrange("p (c w) -> p c w", c=4))
    nc.compile()
    return nc


def _get_kernel():
    if "nc" not in _cache:
        _cache["nc"] = _build_kernel()
        _cache["consts"] = _build_constants()
    return _cache["nc"], _cache["consts"]


def kernel(**inputs):
    nc, consts = _get_kernel()
    Wblk, bprime, WgT, wgb, Wt, rw = _prep_params(inputs)
    feat = np.asarray(inputs["features"], np.float32)
    bf = ml_dtypes.bfloat16

    rcol = np.full((128, 1), rw, np.float32)
    base = {
        "R1": consts["R1"], "R2a": consts["R2a"], "R2b": consts["R2b"],
        "RA1": consts["RA1"], "RA2": consts["RA2"], "RB1": consts["RB1"],
        "RB2": consts["RB2"], "G16": consts["G16"], "E16": consts["E16"],
        "F2": consts["F2"], "E4": consts["E4"], "maskJ": consts["maskJ"],
        "Wblk": Wblk, "bprime": bprime, "WgT": WgT, "wgb": wgb,
        "rcol": rcol,
    }
    in_maps = []
    for k in range(NCORES):
        sl = slice(k * CS, (k + 1) * CS)
        # featf: [h, (b, c, w)]
        ff = feat[:, sl].transpose(2, 0, 1, 3).reshape(128, B * CS * W).copy()
        # featg: [(t, c), (s)] with t = h-half
        fg = feat[k].reshape(C, 2, 64 * 128).transpose(1, 0, 2) \
                    .reshape(128, 64 * 128).copy()
        # ftiles: [J, (f, p), (c, ri, k2)]
        Wts = Wt[:, sl]                                   # [F, CS, H, WF]
        ftiles = np.empty((4, 128, CS * 2 * WF), np.float32)
        for J in range(4):
            blk = Wts[:, :, 32 * J:32 * J + 32, :]        # [F, CS, 32, WF]
            re = blk.real.astype(np.float32)
            im = blk.imag.astype(np.float32)
            # [(f,p), (c, ri, k2)]
            stacked = np.stack([re, im], axis=3)          # [F, CS, 32, 2, WF]
            ftiles[J] = stacked.transpose(0, 2, 1, 3, 4).reshape(128, CS * 2 * WF)
        m = dict(base)
        m["featf"] = ff
        m["featg"] = fg
        m["ftiles"] = ftiles
        in_maps.append(m)

    res = run_bass_kernel_spmd(nc, in_maps, list(range(NCORES)))
    out = np.empty((B, C, H, W), np.float32)
    for k in range(NCORES):
        out[:, k * CS:(k + 1) * CS] = res.results[k]["out"]
    return out


if __name__ == "__main__":
    import jax
    jax.config.update("jax_platforms", "cpu")



# revision 10
# speedup vs baseline: 1.2374x; 1.2374x over previous
"""Trainium2 Bass kernel for nn_FDSM_40295383171690.

Math (identical to the verified baseline):
  gating: GN(concat(x,x)) == concat(GN4(x), GN4(x)); fold gamma/beta into the
          1x1 conv -> W', b'; fold the per-sample GN scale s / shift t into
          the conv as W'' = W' diag(s), b'' = W' t + b' (no xn tensor);
          weights = softmax(wg @ GAP(relu(W'' x + b'')))
  fft:    out = irfft2( rfft2(x)^2 * Wmix ) + r*x
          Wmix[b] = sum_f weights[b,f] * Wsym[f] (Hermitian-symmetrized ds_w)

Sharding: core k = gating for sample k (all C) + FFT branch for channels
[8k,8k+8) of all samples; the [8,4] gating weights are AllGathered on-chip.

Schedule: features/filters ship as bf16 (half DMA); featg lands first so the
gating chain + AllGather start early; phase1 (DFT stages 1-2 + squares, weight
independent) is emitted for all samples before any weight-dependent phase2
work so nothing queues behind the collective.
"""

import numpy as np
import ml_dtypes

import concourse.bass as bass
import concourse.bacc as bacc
import concourse.mybir as mybir
import concourse.tile as tile
from concourse.bass_utils import run_bass_kernel_spmd

dt = mybir.dt
AF = mybir.ActivationFunctionType
ALU = mybir.AluOpType
AX = mybir.AxisListType

B, C, H, W, F = 8, 64, 128, 128, 4
WF = 65
NCORES = 8
CS = C // NCORES
EPS = 1e-5
HW = H * W

_cache = {}
DEBUG = False


def _build_constants():
    h = np.arange(H)
    k1 = np.arange(H)
    w = np.arange(W)
    k2 = np.arange(WF)
    Ch = np.cos(2 * np.pi * np.outer(h, k1) / H).astype(np.float32)
    Sh = np.sin(2 * np.pi * np.outer(h, k1) / H).astype(np.float32)
    Cw = np.cos(2 * np.pi * np.outer(w, k2) / W).astype(np.float32)
    Sw = np.sin(2 * np.pi * np.outer(w, k2) / W).astype(np.float32)
    Cih = (np.cos(2 * np.pi * np.outer(k1, h) / H) / H).astype(np.float32)
    Sih = (np.sin(2 * np.pi * np.outer(k1, h) / H) / H).astype(np.float32)
    cj = np.ones(WF, np.float32)
    cj[1:64] = 2.0
    Gc = (cj[:, None] * np.cos(2 * np.pi * np.outer(k2, w) / W) / W).astype(np.float32)
    Gs = (-cj[:, None] * np.sin(2 * np.pi * np.outer(k2, w) / W) / W).astype(np.float32)

    bf = ml_dtypes.bfloat16
    consts = {
        "R1": np.concatenate([Ch, Sh], 1).astype(bf),
        "R2a": np.concatenate([Cw, -Sw], 1).astype(bf),
        "R2b": np.concatenate([-Sw, -Cw], 1).astype(bf),
        "RA1": np.concatenate([Cih, Sih], 1),
        "RA2": np.concatenate([-Sih, Cih], 1),
        "RB1": Gc.astype(bf),
        "RB2": Gs.astype(bf),
    }
    G16 = np.zeros((128, 16), np.float32)
    E16 = np.zeros((16, 128), np.float32)
    for p in range(128):
        g = (p % 64) // 4
        G16[p, g] = 1.0
        E16[g, p] = 1.0
    F2 = np.zeros((128, 64), np.float32)
    for p in range(128):
        F2[p, p % 64] = 1.0 / HW
    E4 = np.zeros((4, 128), np.float32)
    for p in range(128):
        E4[p // 32, p] = 1.0
    maskJ = np.zeros((128, 4 * 128), np.float32)
    for J in range(4):
        for p in range(128):
            f, pp = p // 32, p % 32
            maskJ[p, 128 * J + 32 * J + pp] = 1.0
    consts.update({"G16": G16, "E16": E16, "F2": F2, "E4": E4,
                   "maskJ": maskJ.astype(bf)})
    return consts


def _prep_params(inputs):
    gamma = np.asarray(inputs["gn_gamma"], np.float64)
    beta = np.asarray(inputs["gn_beta"], np.float64)
    agg_w = np.asarray(inputs["agg_w"], np.float64)
    agg_b = np.asarray(inputs["agg_b"], np.float64)
    wg_w = np.asarray(inputs["wg_w"], np.float64)
    wg_b = np.asarray(inputs["wg_b"], np.float64)

    Wp = agg_w[:, :C] * gamma[None, :C] + agg_w[:, C:] * gamma[None, C:]
    bp = agg_w[:, :C] @ beta[:C] + agg_w[:, C:] @ beta[C:] + agg_b
    Wblk = np.zeros((128, 128), np.float32)
    for t in range(2):
        Wblk[64 * t:64 * t + 64, 64 * t:64 * t + 64] = Wp.T.astype(np.float32)
    bprime = np.zeros((128, 1), np.float32)
    bprime[:64, 0] = bp.astype(np.float32)
    bprime[64:, 0] = bp.astype(np.float32)
    WgT = wg_w.T.astype(np.float32)
    wgb = wg_b.astype(np.float32).reshape(1, 4)

    ds = np.asarray(inputs["ds_w"], np.float64)
    Wc = ds[..., 0] + 1j * ds[..., 1]                     # [F,C,H(k1),WF(k2)]
    rev = (-np.arange(H)) % H
    Wt = Wc.copy()
    for j in (0, WF - 1):
        Wt[..., j] = 0.5 * (Wc[..., j] + np.conj(Wc[:, :, rev, j]))
    rw = float(np.asarray(inputs["residual_weight"]).ravel()[0])
    return Wblk, bprime, WgT, wgb, Wt, rw


def _build_kernel():
    bf16, f32, f32r = dt.bfloat16, dt.float32, dt.float32r

    nc = bacc.Bacc("TRN2", target_bir_lowering=False, debug=False,
                   num_devices=NCORES)

    d = {}
    d["featg"] = nc.dram_tensor("featg", [128, 64 * 128], bf16,
                                kind="ExternalInput").ap()
    d["featf"] = nc.dram_tensor("featf", [128, B * CS * W], bf16,
                                kind="ExternalInput").ap()
    d["ftl"] = nc.dram_tensor("ftl", [4, 128, CS * 2 * WF], bf16,
                              kind="ExternalInput").ap()
    for name, shape, dty in [
        ("R1", [128, 256], bf16), ("R2a", [128, 130], bf16),
        ("R2b", [128, 130], bf16), ("RA1", [128, 256], f32r),
        ("RA2", [128, 256], f32r), ("RB1", [65, 128], bf16),
        ("RB2", [65, 128], bf16), ("maskJ", [128, 512], bf16),
        ("G16", [128, 16], f32), ("E16", [16, 128], f32),
        ("F2", [128, 64], f32), ("E4", [4, 128], f32),
        ("Wblk", [128, 128], bf16), ("bprime", [128, 1], f32),
        ("WgT", [64, 4], f32), ("wgb", [1, 4], f32),
        ("rcol", [128, 1], f32),
    ]:
        d[name] = nc.dram_tensor(name, shape, dty, kind="ExternalInput").ap()
    out_d = nc.dram_tensor("out", [B, CS, H, W], f32, kind="ExternalOutput").ap()
    if DEBUG:
        dbg = {
            "d_uv": nc.dram_tensor("d_uv", [128, 2048], f32, kind="ExternalOutput").ap(),
            "d_X": nc.dram_tensor("d_X", [128, 1040], f32, kind="ExternalOutput").ap(),
            "d_Sr": nc.dram_tensor("d_Sr", [128, 520], f32, kind="ExternalOutput").ap(),
            "d_Si": nc.dram_tensor("d_Si", [128, 520], f32, kind="ExternalOutput").ap(),
            "d_Wm": nc.dram_tensor("d_Wm", [128, 1040], f32, kind="ExternalOutput").ap(),
            "d_Dr": nc.dram_tensor("d_Dr", [128, 520], f32, kind="ExternalOutput").ap(),
            "d_Di": nc.dram_tensor("d_Di", [128, 520], f32, kind="ExternalOutput").ap(),
            "d_z2": nc.dram_tensor("d_z2", [65, 2048], f32, kind="ExternalOutput").ap(),
            "d_wcol": nc.dram_tensor("d_wcol", [128, 8], f32, kind="ExternalOutput").ap(),
            "d_stats": nc.dram_tensor("d_stats", [128, 4], f32, kind="ExternalOutput").ap(),
            "d_nstat": nc.dram_tensor("d_nstat", [128, 2], f32, kind="ExternalOutput").ap(),
            "d_bias2": nc.dram_tensor("d_bias2", [128, 1], f32, kind="ExternalOutput").ap(),
            "d_gap": nc.dram_tensor("d_gap", [128, 16], f32, kind="ExternalOutput").ap(),
            "d_pooled": nc.dram_tensor("d_pooled", [64, 1], f32, kind="ExternalOutput").ap(),
            "d_wrow": nc.dram_tensor("d_wrow", [1, 4], f32, kind="ExternalOutput").ap(),
            "d_gs": nc.dram_tensor("d_gs", [16, 8], f32, kind="ExternalOutput").ap(),
        }

    with tile.TileContext(nc) as tc:
        with (
            tc.tile_pool(name="consts", bufs=1) as cp,
            tc.tile_pool(name="feat", bufs=1) as fp,
            tc.tile_pool(name="gate", bufs=1) as gp,
            tc.tile_pool(name="uvp", bufs=2) as uvp,
            tc.tile_pool(name="xp", bufs=2) as xp,
            tc.tile_pool(name="tmp", bufs=2) as tp,
            tc.tile_pool(name="sqp", bufs=1) as sqp,
            tc.tile_pool(name="wmp", bufs=2) as wmp,
            tc.tile_pool(name="dp", bufs=2) as dp_,
            tc.tile_pool(name="z2p", bufs=2) as z2p,
            tc.tile_pool(name="outp", bufs=3) as op_,
            tc.tile_pool(name="ps1", bufs=2, space="PSUM") as ps1,
            tc.tile_pool(name="ps2", bufs=2, space="PSUM") as ps2,
            tc.tile_pool(name="psm", bufs=2, space="PSUM") as psm,
            tc.tile_pool(name="psb", bufs=2, space="PSUM") as psb,
            tc.tile_pool(name="dram", bufs=1, space="DRAM") as dr,
        ):
            # ---------------- input DMAs ----------------
            # SP queue: featg (gating-critical, first), then featf, ftl.
            featg = fp.tile([128, 64 * 128], bf16, tag="featg")
            nc.sync.dma_start(featg[:, 0:4096], d["featg"][:, 0:4096])
            nc.sync.dma_start(featg[:, 4096:8192], d["featg"][:, 4096:8192])
            featb = []
            for b in range(B):
                t = fp.tile([128, CS * W], bf16, tag=f"featb{b}")
                nc.sync.dma_start(t[:], d["featf"][:, b * CS * W:(b + 1) * CS * W])
                featb.append(t)
            ftl = []
            for J in range(4):
                t = fp.tile([128, CS * 2 * WF], bf16, tag=f"ftl{J}")
                nc.sync.dma_start(t[:], d["ftl"][J])
                ftl.append(t)
            # Pool queue (cheap issue, idle early): all small consts.
            ct = {}
            for name in ["G16", "E16", "F2", "E4", "Wblk", "bprime", "WgT",
                         "wgb", "rcol", "R1", "R2a", "R2b", "maskJ",
                         "RA1", "RA2", "RB1", "RB2"]:
                t = cp.tile(list(d[name].shape), d[name].dtype, tag=name)
                nc.gpsimd.dma_start(t[:], d[name][:])
                ct[name] = t

            # ---------------- gating (sample = core id) ----------------
            stats = gp.tile([128, 4], f32, tag="stats")
            scr_q = gp.tile([128, 4096], bf16, tag="scr_q")
            for hh in range(2):
                sl = slice(hh * 4096, (hh + 1) * 4096)
                nc.vector.tensor_reduce(stats[:, hh:hh + 1], featg[:, sl],
                                        AX.X, ALU.add)
                nc.scalar.activation(scr_q[:], featg[:, sl], AF.Square,
                                     accum_out=stats[:, 2 + hh:3 + hh])
            gstat = psb.tile([16, 4], f32, tag="psb")
            nc.tensor.matmul(gstat[:], ct["G16"][:], stats[:])
            gsf = gp.tile([16, 4], f32, tag="gsf")
            nc.scalar.copy(gsf[:], gstat[:])
            gs = gp.tile([16, 8], f32, tag="gs")
            nc.vector.tensor_add(gs[:, 0:1], gsf[:, 0:1], gsf[:, 1:2])
            nc.vector.tensor_add(gs[:, 1:2], gsf[:, 2:3], gsf[:, 3:4])
            nc.scalar.mul(gs[:, 2:3], gs[:, 0:1], 1.0 / (4 * HW))   # mean
            nc.scalar.mul(gs[:, 3:4], gs[:, 1:2], 1.0 / (4 * HW))   # E[x^2]
            nc.vector.tensor_mul(gs[:, 4:5], gs[:, 2:3], gs[:, 2:3])
            nc.vector.tensor_sub(gs[:, 5:6], gs[:, 3:4], gs[:, 4:5])  # var
            epst = gp.tile([16, 1], f32, tag="epst")
            nc.vector.memset(epst[:], EPS)
            nc.scalar.activation(gs[:, 6:7], gs[:, 5:6], AF.Sqrt,
                                 bias=epst[:, 0:1])
            nc.vector.reciprocal(gs[:, 7:8], gs[:, 6:7])              # rstd
            gs2 = gp.tile([16, 2], f32, tag="gs2")
            nc.vector.tensor_mul(gs2[:, 0:1], gs[:, 2:3], gs[:, 7:8])
            nc.vector.tensor_scalar_mul(gs2[:, 0:1], gs2[:, 0:1], -1.0)
            nc.vector.tensor_copy(gs2[:, 1:2], gs[:, 7:8])
            pstat = psb.tile([128, 2], f32, tag="psb")
            nc.tensor.matmul(pstat[:], ct["E16"][:], gs2[:])
            nstat = gp.tile([128, 2], f32, tag="nstat")
            nc.scalar.copy(nstat[:], pstat[:])
            tvecb = gp.tile([128, 1], bf16, tag="tvecb")
            nc.vector.tensor_copy(tvecb[:], nstat[:, 0:1])
            wblk2 = gp.tile([128, 128], bf16, tag="wblk2")
            nc.scalar.activation(wblk2[:], ct["Wblk"][:], AF.Identity,
                                 scale=nstat[:, 1:2])
            pb2 = psb.tile([128, 1], f32, tag="psb")
            nc.tensor.matmul(pb2[:], ct["Wblk"][:], tvecb[:])
            bias2 = gp.tile([128, 1], f32, tag="bias2")
            nc.vector.tensor_add(bias2[:], pb2[:], ct["bprime"][:])
            nbias2 = gp.tile([128, 1], f32, tag="nbias2")
            nc.vector.tensor_scalar_mul(nbias2[:], bias2[:], -1.0)

            gap = gp.tile([128, 16], f32, tag="gap")
            scrA = gp.tile([128, 512], bf16, tag="scrA")
            scrD = gp.tile([128, 512], bf16, tag="scrD")
            for j in range(16):
                yp = psb.tile([128, 512], f32, tag="psb")
                nc.tensor.matmul(yp[:], wblk2[:],
                                 featg[:, j * 512:(j + 1) * 512])
                if j % 2 == 0:
                    nc.scalar.activation(scrA[:], yp[:], AF.Relu,
                                         bias=bias2[:, 0:1],
                                         accum_out=gap[:, j:j + 1])
                else:
                    nc.vector.scalar_tensor_tensor(
                        scrD[:], yp[:], nbias2[:, 0:1],
                        bias2[:, 0:1].broadcast_to((128, 512)),
                        ALU.max, ALU.add, accum_out=gap[:, j:j + 1])
            gsum = gp.tile([128, 1], f32, tag="gsum")
            nc.vector.tensor_reduce(gsum[:], gap[:], AX.X, ALU.add)
            ppool = psb.tile([64, 1], f32, tag="psb")
            nc.tensor.matmul(ppool[:], ct["F2"][:], gsum[:])
            pooled = gp.tile([64, 1], f32, tag="pooled")
            nc.scalar.copy(pooled[:], ppool[:])
            plog = psb.tile([1, 4], f32, tag="psb")
            nc.tensor.matmul(plog[:], pooled[:], ct["WgT"][:])
            logit = gp.tile([1, 8], f32, tag="logit")
            nc.vector.memset(logit[:], 0.0)
            nc.vector.tensor_add(logit[:, 0:4], plog[:], ct["wgb"][:])
            nc.vector.tensor_reduce(logit[:, 4:5], logit[:, 0:4], AX.X, ALU.max)
            nc.vector.tensor_scalar(logit[:, 0:4], logit[:, 0:4],
                                    logit[:, 4:5], None, ALU.subtract)
            wrow = gp.tile([1, 4], f32, tag="wrow")
            nc.scalar.activation(wrow[:], logit[:, 0:4], AF.Exp,
                                 accum_out=logit[:, 5:6])
            nc.vector.reciprocal(logit[:, 6:7], logit[:, 5:6])
            nc.vector.tensor_scalar(wrow[:], wrow[:], logit[:, 6:7], None,
                                    ALU.mult)
            if DEBUG:
                nc.sync.dma_start(dbg["d_stats"][:], stats[:])
                nc.sync.dma_start(dbg["d_nstat"][:], nstat[:])
                nc.sync.dma_start(dbg["d_bias2"][:], bias2[:])
                nc.sync.dma_start(dbg["d_gap"][:], gap[:])
                nc.sync.dma_start(dbg["d_pooled"][:], pooled[:])
                nc.sync.dma_start(dbg["d_wrow"][:], wrow[:])
                nc.sync.dma_start(dbg["d_gs"][:], gs[:])
            ag_in = dr.tile([1, 4], f32)
            ag_out = dr.tile([8, 4], f32)
            nc.sync.dma_start(ag_in[:], wrow[:])
            nc.gpsimd.collective_compute(
                "AllGather", ALU.bypass, ins=[ag_in.opt()],
                outs=[ag_out.opt()],
                replica_groups=[list(range(NCORES))],
            )

            # ---------------- phase 1: DFT + squares (no weights) -----------
            Srs, Sis = [], []
            for b in range(B):
                fb = featb[b]
                uvb = uvp.tile([128, 2048], bf16, tag="uvb")
                p2s = []
                for cc in range(4):
                    p1 = ps1.tile([128, 512], f32, tag="p1")
                    for j in range(2):
                        c = 2 * cc + j
                        nc.tensor.matmul(p1[:, j * 256:(j + 1) * 256],
                                         fb[:, c * 128:(c + 1) * 128],
                                         ct["R1"][:])
                    if cc % 2 == 0:
                        nc.vector.tensor_copy(uvb[:, cc * 512:(cc + 1) * 512],
                                              p1[:])
                    else:
                        nc.scalar.copy(uvb[:, cc * 512:(cc + 1) * 512], p1[:])
                    p2 = ps2.tile([128, 260], f32, tag="p2")
                    for j in range(2):
                        base = cc * 512 + j * 256
                        nc.tensor.matmul(p2[:, j * 130:(j + 1) * 130],
                                         uvb[:, base:base + 128],
                                         ct["R2a"][:], start=True, stop=False)
                        nc.tensor.matmul(p2[:, j * 130:(j + 1) * 130],
                                         uvb[:, base + 128:base + 256],
                                         ct["R2b"][:], start=False, stop=True)
                    p2s.append(p2)
                X = xp.tile([128, 1040], f32, tag="X")
                Xv = X[:].rearrange("p (r c x) -> p r c x", r=2, c=8)
                for cc in range(4):
                    src = p2s[cc][:].rearrange("p (j r x) -> p r j x", j=2, r=2)
                    dst = Xv[:, :, 2 * cc:2 * cc + 2, :]
                    if cc % 2 == 0:
                        nc.scalar.copy(dst, src)
                    else:
                        nc.vector.tensor_copy(dst, src)
                Xr, Xi = X[:, 0:520], X[:, 520:1040]
                t1 = tp.tile([128, 520], f32, tag="t1")
                t2 = tp.tile([128, 520], f32, tag="t2")
                Sr = sqp.tile([128, 520], f32, tag=f"Sr{b}")
                Si = sqp.tile([128, 520], f32, tag=f"Si{b}")
                nc.vector.tensor_add(t1[:], Xr, Xi)
                nc.vector.scalar_tensor_tensor(Si[:], Xr, 2.0, Xi,
                                               ALU.mult, ALU.mult)
                if b < 4:
                    nc.vector.tensor_sub(t2[:], Xr, Xi)
                    nc.vector.tensor_mul(Sr[:], t1[:], t2[:])
                else:
                    nc.gpsimd.tensor_sub(t2[:], Xr, Xi)
                    nc.gpsimd.tensor_mul(Sr[:], t1[:], t2[:])
                Srs.append(Sr)
                Sis.append(Si)
                if DEBUG and b == 0:
                    for nm, t in [("d_uv", uvb), ("d_X", X), ("d_Sr", Sr), ("d_Si", Si)]:
                        tf = gp.tile(list(t.shape), f32, tag="dump" + nm)
                        nc.vector.tensor_copy(tf[:], t[:])
                        nc.sync.dma_start(dbg[nm][:], tf[:])

            # ---------------- gathered weights -> per-sample masks ----------
            wT = gp.tile([4, 8], f32, tag="wT")
            nc.sync.dma_start(wT[:], ag_out[:].rearrange("b f -> f b"))
            pwcol = psb.tile([128, 8], f32, tag="psb")
            nc.tensor.matmul(pwcol[:], ct["E4"][:], wT[:])
            wcol = gp.tile([128, 8], f32, tag="wcol")
            nc.scalar.copy(wcol[:], pwcol[:])
            if DEBUG:
                nc.sync.dma_start(dbg["d_wcol"][:], wcol[:])
            wpat = []
            for b in range(B):
                t = gp.tile([128, 512], bf16, tag=f"wpat{b}")
                nc.vector.tensor_scalar(t[:], ct["maskJ"][:],
                                        wcol[:, b:b + 1], None, ALU.mult)
                wpat.append(t)

            # ---------------- phase 2: mix, product, inverse DFT ------------
            for b in range(B):
                fb = featb[b]
                Wm = wmp.tile([128, 1040], f32, tag="Wm")
                Wmv = Wm[:].rearrange("p (r c x) -> p r c x", r=2, c=8)
                for cc in range(4):
                    pm = psm.tile([128, 260], f32, tag="psm")
                    for J in range(4):
                        nc.tensor.matmul(
                            pm[:], wpat[b][:, J * 128:(J + 1) * 128],
                            ftl[J][:, cc * 260:(cc + 1) * 260],
                            start=(J == 0), stop=(J == 3))
                    src = pm[:].rearrange("p (j r x) -> p r j x", j=2, r=2)
                    dst = Wmv[:, :, 2 * cc:2 * cc + 2, :]
                    if cc % 2 == 0:
                        nc.scalar.copy(dst, src)
                    else:
                        nc.vector.tensor_copy(dst, src)
                m1 = tp.tile([128, 520], f32, tag="m1")
                m2 = tp.tile([128, 520], f32, tag="m2")
                m3 = tp.tile([128, 520], f32, tag="m3")
                m4 = tp.tile([128, 520], f32, tag="m4")
                Dr = dp_.tile([128, 520], f32r, tag="Dr")
                Di = dp_.tile([128, 520], f32r, tag="Di")
                nc.vector.tensor_mul(m1[:], Srs[b][:], Wm[:, 0:520])
                nc.gpsimd.tensor_mul(m2[:], Sis[b][:], Wm[:, 520:1040])
                nc.vector.tensor_mul(m3[:], Srs[b][:], Wm[:, 520:1040])
                nc.gpsimd.tensor_mul(m4[:], Sis[b][:], Wm[:, 0:520])
                nc.vector.tensor_sub(Dr[:], m1[:], m2[:])
                nc.gpsimd.tensor_add(Di[:], m3[:], m4[:])
                if DEBUG and b == 0:
                    for nm, t in [("d_Wm", Wm), ("d_Dr", Dr), ("d_Di", Di)]:
                        tf = gp.tile(list(t.shape), f32, tag="dump" + nm)
                        nc.vector.tensor_copy(tf[:], t[:])
                        nc.sync.dma_start(dbg[nm][:], tf[:])
                z2b = z2p.tile([65, 2048], bf16, tag="z2b")
                for cc in range(4):
                    pA = psm.tile([65, 512], f32, tag="psm")
                    for j in range(2):
                        q = 2 * cc + j
                        nc.tensor.matmul(pA[:, j * 256:(j + 1) * 256],
                                         Dr[:, q * 65:(q + 1) * 65],
                                         ct["RA1"][:], start=True, stop=False)
                        nc.tensor.matmul(pA[:, j * 256:(j + 1) * 256],
                                         Di[:, q * 65:(q + 1) * 65],
                                         ct["RA2"][:], start=False, stop=True)
                    if cc % 2 == 0:
                        nc.vector.tensor_copy(z2b[:, cc * 512:(cc + 1) * 512],
                                              pA[:])
                    else:
                        nc.scalar.copy(z2b[:, cc * 512:(cc + 1) * 512], pA[:])
                if DEBUG and b == 0:
                    tf = gp.tile([65, 2048], f32, tag="dumpd_z2")
                    nc.vector.tensor_copy(tf[:], z2b[:])
                    nc.sync.dma_start(dbg["d_z2"][:], tf[:])
                for g in range(2):
                    pB = psb.tile([128, 512], f32, tag="psb")
                    for j in range(4):
                        q = 4 * g + j
                        nc.tensor.matmul(pB[:, j * 128:(j + 1) * 128],
                                         z2b[:, q * 256:q * 256 + 128],
                                         ct["RB1"][:], start=True, stop=False)
                        nc.tensor.matmul(pB[:, j * 128:(j + 1) * 128],
                                         z2b[:, q * 256 + 128:(q + 1) * 256],
                                         ct["RB2"][:], start=False, stop=True)
                    ot = op_.tile([128, 512], f32, tag="ot")
                    nc.vector.scalar_tensor_tensor(
                        ot[:], fb[:, g * 512:(g + 1) * 512],
                        ct["rcol"][:, 0:1], pB[:], ALU.mult, ALU.add)
                    nc.gpsimd.dma_start(
                        out_d[b, g * 4:(g + 1) * 4].rearrange("c h w -> h c w"),
                        ot[:].rearrange("p (c w) -> p c w", c=4))
    nc.compile()
    return nc


def _get_kernel():
    if "nc" not in _cache:
        _cache["nc"] = _build_kernel()
        _cache["consts"] = _build_constants()
    return _cache["nc"], _cache["consts"]


def kernel(**inputs):
    nc, consts = _get_kernel()
    Wblk, bprime, WgT, wgb, Wt, rw = _prep_params(inputs)
    feat = np.asarray(inputs["features"], np.float32)
    bf = ml_dtypes.bfloat16

    rcol = np.full((128, 1), rw, np.float32)
    base = {
        "R1": consts["R1"], "R2a": consts["R2a"], "R2b": consts["R2b"],
        "RA1": consts["RA1"], "RA2": consts["RA2"], "RB1": consts["RB1"],
        "RB2": consts["RB2"], "G16": consts["G16"], "E16": consts["E16"],
        "F2": consts["F2"], "E4": consts["E4"], "maskJ": consts["maskJ"],
        "Wblk": Wblk.astype(bf), "bprime": bprime, "WgT": WgT, "wgb": wgb,
        "rcol": rcol,
    }
    in_maps = []
    for k in range(NCORES):
        sl = slice(k * CS, (k + 1) * CS)
        # featf: [h, (b, c, w)]
        ff = feat[:, sl].transpose(2, 0, 1, 3).reshape(128, B * CS * W)
        # featg: [(t, c), (s)] with t = h-half
        fg = feat[k].reshape(C, 2, 64 * 128).transpose(1, 0, 2) \
                    .reshape(128, 64 * 128)
        # ftl: [J, (f, p), (c, ri, k2)]
        Wts = Wt[:, sl]                                   # [F, CS, H, WF]
        ftiles = np.empty((4, 128, CS * 2 * WF), np.float32)
        for J in range(4):
            blk = Wts[:, :, 32 * J:32 * J + 32, :]        # [F, CS, 32, WF]
            re = blk.real.astype(np.float32)
            im = blk.imag.astype(np.float32)
            stacked = np.stack([re, im], axis=3)          # [F, CS, 32, 2, WF]
            ftiles[J] = stacked.transpose(0, 2, 1, 3, 4).reshape(128, CS * 2 * WF)
        m = dict(base)
        m["featf"] = ff.astype(bf)
        m["featg"] = fg.astype(bf)
        m["ftl"] = ftiles.astype(bf)
        in_maps.append(m)

    res = run_bass_kernel_spmd(nc, in_maps, list(range(NCORES)))
    _cache["res"] = res
    out = np.empty((B, C, H, W), np.float32)
    for k in range(NCORES):
        out[:, k * CS:(k + 1) * CS] = res.results[k]["out"]
    return out


# revision 11
# speedup vs baseline: 1.2426x; 1.0042x over previous
"""Trainium2 Bass kernel for nn_FDSM_40295383171690.

Math (identical to the verified baseline):
  gating: GN(concat(x,x)) == concat(GN4(x), GN4(x)); fold gamma/beta into the
          1x1 conv -> W', b'; fold the per-sample GN scale s / shift t into
          the conv as W'' = W' diag(s), b'' = W' t + b' (no xn tensor);
          weights = softmax(wg @ GAP(relu(W'' x + b'')))
  fft:    out = irfft2( rfft2(x)^2 * Wmix ) + r*x
          Wmix[b] = sum_f weights[b,f] * Wsym[f] (Hermitian-symmetrized ds_w)

Sharding: core k = gating for sample k (all C) + FFT branch for channels
[8k,8k+8) of all samples; the [8,4] gating weights are AllGathered on-chip.

Schedule: features/filters ship as bf16 (half DMA); featg lands first so the
gating chain + AllGather start early; phase1 (DFT stages 1-2 + squares, weight
independent) is emitted for all samples before any weight-dependent phase2
work so nothing queues behind the collective.
"""

import numpy as np
import ml_dtypes

import concourse.bass as bass
import concourse.bacc as bacc
import concourse.mybir as mybir
import concourse.tile as tile
from concourse.bass_utils import run_bass_kernel_spmd

dt = mybir.dt
AF = mybir.ActivationFunctionType
ALU = mybir.AluOpType
AX = mybir.AxisListType

B, C, H, W, F = 8, 64, 128, 128, 4
WF = 65
NCORES = 8
CS = C // NCORES
EPS = 1e-5
HW = H * W

_cache = {}
DEBUG = False


def _build_constants():
    h = np.arange(H)
    k1 = np.arange(H)
    w = np.arange(W)
    k2 = np.arange(WF)
    Ch = np.cos(2 * np.pi * np.outer(h, k1) / H).astype(np.float32)
    Sh = np.sin(2 * np.pi * np.outer(h, k1) / H).astype(np.float32)
    Cw = np.cos(2 * np.pi * np.outer(w, k2) / W).astype(np.float32)
    Sw = np.sin(2 * np.pi * np.outer(w, k2) / W).astype(np.float32)
    Cih = (np.cos(2 * np.pi * np.outer(k1, h) / H) / H).astype(np.float32)
    Sih = (np.sin(2 * np.pi * np.outer(k1, h) / H) / H).astype(np.float32)
    cj = np.ones(WF, np.float32)
    cj[1:64] = 2.0
    Gc = (cj[:, None] * np.cos(2 * np.pi * np.outer(k2, w) / W) / W).astype(np.float32)
    Gs = (-cj[:, None] * np.sin(2 * np.pi * np.outer(k2, w) / W) / W).astype(np.float32)

    bf = ml_dtypes.bfloat16
    consts = {
        "R1": np.concatenate([Ch, Sh], 1).astype(bf),
        "R2a": np.concatenate([Cw, -Sw], 1).astype(bf),
        "R2b": np.concatenate([-Sw, -Cw], 1).astype(bf),
        "RA1": np.concatenate([Cih, Sih], 1).astype(bf),
        "RA2": np.concatenate([-Sih, Cih], 1).astype(bf),
        "RB1": Gc.astype(bf),
        "RB2": Gs.astype(bf),
    }
    G16 = np.zeros((128, 16), np.float32)
    E16 = np.zeros((16, 128), np.float32)
    for p in range(128):
        g = (p % 64) // 4
        G16[p, g] = 1.0
        E16[g, p] = 1.0
    F2 = np.zeros((128, 64), np.float32)
    for p in range(128):
        F2[p, p % 64] = 1.0 / HW
    E4 = np.zeros((4, 128), np.float32)
    for p in range(128):
        E4[p // 32, p] = 1.0
    maskJ = np.zeros((128, 4 * 128), np.float32)
    for J in range(4):
        for p in range(128):
            f, pp = p // 32, p % 32
            maskJ[p, 128 * J + 32 * J + pp] = 1.0
    consts.update({"G16": G16, "E16": E16, "F2": F2, "E4": E4,
                   "maskJ": maskJ.astype(bf)})
    return consts


def _prep_params(inputs):
    gamma = np.asarray(inputs["gn_gamma"], np.float64)
    beta = np.asarray(inputs["gn_beta"], np.float64)
    agg_w = np.asarray(inputs["agg_w"], np.float64)
    agg_b = np.asarray(inputs["agg_b"], np.float64)
    wg_w = np.asarray(inputs["wg_w"], np.float64)
    wg_b = np.asarray(inputs["wg_b"], np.float64)

    Wp = agg_w[:, :C] * gamma[None, :C] + agg_w[:, C:] * gamma[None, C:]
    bp = agg_w[:, :C] @ beta[:C] + agg_w[:, C:] @ beta[C:] + agg_b
    Wblk = np.zeros((128, 128), np.float32)
    for t in range(2):
        Wblk[64 * t:64 * t + 64, 64 * t:64 * t + 64] = Wp.T.astype(np.float32)
    bprime = np.zeros((128, 1), np.float32)
    bprime[:64, 0] = bp.astype(np.float32)
    bprime[64:, 0] = bp.astype(np.float32)
    WgT = wg_w.T.astype(np.float32)
    wgb = wg_b.astype(np.float32).reshape(1, 4)

    ds = np.asarray(inputs["ds_w"], np.float64)
    Wc = ds[..., 0] + 1j * ds[..., 1]                     # [F,C,H(k1),WF(k2)]
    rev = (-np.arange(H)) % H
    Wt = Wc.copy()
    for j in (0, WF - 1):
        Wt[..., j] = 0.5 * (Wc[..., j] + np.conj(Wc[:, :, rev, j]))
    rw = float(np.asarray(inputs["residual_weight"]).ravel()[0])
    return Wblk, bprime, WgT, wgb, Wt, rw


def _build_kernel():
    bf16, f32, f32r = dt.bfloat16, dt.float32, dt.float32r

    nc = bacc.Bacc("TRN2", target_bir_lowering=False, debug=False,
                   num_devices=NCORES)

    d = {}
    d["featg"] = nc.dram_tensor("featg", [128, 64 * 128], bf16,
                                kind="ExternalInput").ap()
    d["featf"] = nc.dram_tensor("featf", [128, B * CS * W], bf16,
                                kind="ExternalInput").ap()
    d["ftl"] = nc.dram_tensor("ftl", [4, 128, CS * 2 * WF], bf16,
                              kind="ExternalInput").ap()
    for name, shape, dty in [
        ("R1", [128, 256], bf16), ("R2a", [128, 130], bf16),
        ("R2b", [128, 130], bf16), ("RA1", [128, 256], bf16),
        ("RA2", [128, 256], bf16), ("RB1", [65, 128], bf16),
        ("RB2", [65, 128], bf16), ("maskJ", [128, 512], bf16),
        ("G16", [128, 16], f32), ("E16", [16, 128], f32),
        ("F2", [128, 64], f32), ("E4", [4, 128], f32),
        ("Wblk", [128, 128], bf16), ("bprime", [128, 1], f32),
        ("WgT", [64, 4], f32), ("wgb", [1, 4], f32),
        ("rcol", [128, 1], f32),
    ]:
        d[name] = nc.dram_tensor(name, shape, dty, kind="ExternalInput").ap()
    out_d = nc.dram_tensor("out", [B, CS, H, W], f32, kind="ExternalOutput").ap()
    if DEBUG:
        dbg = {
            "d_uv": nc.dram_tensor("d_uv", [128, 2048], f32, kind="ExternalOutput").ap(),
            "d_X": nc.dram_tensor("d_X", [128, 1040], f32, kind="ExternalOutput").ap(),
            "d_Sr": nc.dram_tensor("d_Sr", [128, 520], f32, kind="ExternalOutput").ap(),
            "d_Si": nc.dram_tensor("d_Si", [128, 520], f32, kind="ExternalOutput").ap(),
            "d_Wm": nc.dram_tensor("d_Wm", [128, 1040], f32, kind="ExternalOutput").ap(),
            "d_Dr": nc.dram_tensor("d_Dr", [128, 520], f32, kind="ExternalOutput").ap(),
            "d_Di": nc.dram_tensor("d_Di", [128, 520], f32, kind="ExternalOutput").ap(),
            "d_z2": nc.dram_tensor("d_z2", [65, 2048], f32, kind="ExternalOutput").ap(),
            "d_wcol": nc.dram_tensor("d_wcol", [128, 8], f32, kind="ExternalOutput").ap(),
            "d_stats": nc.dram_tensor("d_stats", [128, 4], f32, kind="ExternalOutput").ap(),
            "d_nstat": nc.dram_tensor("d_nstat", [128, 2], f32, kind="ExternalOutput").ap(),
            "d_bias2": nc.dram_tensor("d_bias2", [128, 1], f32, kind="ExternalOutput").ap(),
            "d_gap": nc.dram_tensor("d_gap", [128, 16], f32, kind="ExternalOutput").ap(),
            "d_pooled": nc.dram_tensor("d_pooled", [64, 1], f32, kind="ExternalOutput").ap(),
            "d_wrow": nc.dram_tensor("d_wrow", [1, 4], f32, kind="ExternalOutput").ap(),
            "d_gs": nc.dram_tensor("d_gs", [16, 8], f32, kind="ExternalOutput").ap(),
        }

    with tile.TileContext(nc) as tc:
        with (
            tc.tile_pool(name="consts", bufs=1) as cp,
            tc.tile_pool(name="feat", bufs=1) as fp,
            tc.tile_pool(name="gate", bufs=1) as gp,
            tc.tile_pool(name="uvp", bufs=2) as uvp,
            tc.tile_pool(name="xp", bufs=2) as xp,
            tc.tile_pool(name="tmp", bufs=2) as tp,
            tc.tile_pool(name="sqp", bufs=1) as sqp,
            tc.tile_pool(name="wmp", bufs=2) as wmp,
            tc.tile_pool(name="dp", bufs=2) as dp_,
            tc.tile_pool(name="z2p", bufs=2) as z2p,
            tc.tile_pool(name="outp", bufs=3) as op_,
            tc.tile_pool(name="ps1", bufs=2, space="PSUM") as ps1,
            tc.tile_pool(name="ps2", bufs=2, space="PSUM") as ps2,
            tc.tile_pool(name="psm", bufs=2, space="PSUM") as psm,
            tc.tile_pool(name="psb", bufs=2, space="PSUM") as psb,
            tc.tile_pool(name="dram", bufs=1, space="DRAM") as dr,
        ):
            # ---------------- input DMAs ----------------
            # SP queue: featg (gating-critical, first), then featf, ftl.
            featg = fp.tile([128, 64 * 128], bf16, tag="featg")
            nc.sync.dma_start(featg[:, 0:4096], d["featg"][:, 0:4096])
            nc.sync.dma_start(featg[:, 4096:8192], d["featg"][:, 4096:8192])
            featb = []
            for b in range(B):
                t = fp.tile([128, CS * W], bf16, tag=f"featb{b}")
                nc.sync.dma_start(t[:], d["featf"][:, b * CS * W:(b + 1) * CS * W])
                featb.append(t)
            ftl = []
            for J in range(4):
                t = fp.tile([128, CS * 2 * WF], bf16, tag=f"ftl{J}")
                nc.sync.dma_start(t[:], d["ftl"][J])
                ftl.append(t)
            # Pool queue (cheap issue, idle early): all small consts.
            ct = {}
            for name in ["G16", "E16", "F2", "E4", "Wblk", "bprime", "WgT",
                         "wgb", "rcol", "R1", "R2a", "R2b", "maskJ",
                         "RA1", "RA2", "RB1", "RB2"]:
                t = cp.tile(list(d[name].shape), d[name].dtype, tag=name)
                nc.gpsimd.dma_start(t[:], d[name][:])
                ct[name] = t

            # ---------------- gating (sample = core id) ----------------
            stats = gp.tile([128, 4], f32, tag="stats")
            scr_q = gp.tile([128, 4096], bf16, tag="scr_q")
            for hh in range(2):
                sl = slice(hh * 4096, (hh + 1) * 4096)
                nc.vector.tensor_reduce(stats[:, hh:hh + 1], featg[:, sl],
                                        AX.X, ALU.add)
                nc.scalar.activation(scr_q[:], featg[:, sl], AF.Square,
                                     accum_out=stats[:, 2 + hh:3 + hh])
            gstat = psb.tile([16, 4], f32, tag="psb")
            nc.tensor.matmul(gstat[:], ct["G16"][:], stats[:])
            gsf = gp.tile([16, 4], f32, tag="gsf")
            nc.scalar.copy(gsf[:], gstat[:])
            gs = gp.tile([16, 8], f32, tag="gs")
            nc.vector.tensor_add(gs[:, 0:1], gsf[:, 0:1], gsf[:, 1:2])
            nc.vector.tensor_add(gs[:, 1:2], gsf[:, 2:3], gsf[:, 3:4])
            nc.scalar.mul(gs[:, 2:3], gs[:, 0:1], 1.0 / (4 * HW))   # mean
            nc.scalar.mul(gs[:, 3:4], gs[:, 1:2], 1.0 / (4 * HW))   # E[x^2]
            nc.vector.tensor_mul(gs[:, 4:5], gs[:, 2:3], gs[:, 2:3])
            nc.vector.tensor_sub(gs[:, 5:6], gs[:, 3:4], gs[:, 4:5])  # var
            epst = gp.tile([16, 1], f32, tag="epst")
            nc.vector.memset(epst[:], EPS)
            nc.scalar.activation(gs[:, 6:7], gs[:, 5:6], AF.Sqrt,
                                 bias=epst[:, 0:1])
            nc.vector.reciprocal(gs[:, 7:8], gs[:, 6:7])              # rstd
            gs2 = gp.tile([16, 2], f32, tag="gs2")
            nc.vector.tensor_mul(gs2[:, 0:1], gs[:, 2:3], gs[:, 7:8])
            nc.vector.tensor_scalar_mul(gs2[:, 0:1], gs2[:, 0:1], -1.0)
            nc.vector.tensor_copy(gs2[:, 1:2], gs[:, 7:8])
            pstat = psb.tile([128, 2], f32, tag="psb")
            nc.tensor.matmul(pstat[:], ct["E16"][:], gs2[:])
            nstat = gp.tile([128, 2], f32, tag="nstat")
            nc.scalar.copy(nstat[:], pstat[:])
            tvecb = gp.tile([128, 1], bf16, tag="tvecb")
            nc.vector.tensor_copy(tvecb[:], nstat[:, 0:1])
            wblk2 = gp.tile([128, 128], bf16, tag="wblk2")
            nc.scalar.activation(wblk2[:], ct["Wblk"][:], AF.Identity,
                                 scale=nstat[:, 1:2])
            pb2 = psb.tile([128, 1], f32, tag="psb")
            nc.tensor.matmul(pb2[:], ct["Wblk"][:], tvecb[:])
            bias2 = gp.tile([128, 1], f32, tag="bias2")
            nc.vector.tensor_add(bias2[:], pb2[:], ct["bprime"][:])
            nbias2 = gp.tile([128, 1], f32, tag="nbias2")
            nc.vector.tensor_scalar_mul(nbias2[:], bias2[:], -1.0)

            gap = gp.tile([128, 16], f32, tag="gap")
            scrA = gp.tile([128, 512], bf16, tag="scrA")
            scrD = gp.tile([128, 512], bf16, tag="scrD")
            for j in range(16):
                yp = psb.tile([128, 512], f32, tag="psb")
                nc.tensor.matmul(yp[:], wblk2[:],
                                 featg[:, j * 512:(j + 1) * 512])
                if j % 2 == 0:
                    nc.scalar.activation(scrA[:], yp[:], AF.Relu,
                                         bias=bias2[:, 0:1],
                                         accum_out=gap[:, j:j + 1])
                else:
                    nc.vector.scalar_tensor_tensor(
                        scrD[:], yp[:], nbias2[:, 0:1],
                        bias2[:, 0:1].broadcast_to((128, 512)),
                        ALU.max, ALU.add, accum_out=gap[:, j:j + 1])
            gsum = gp.tile([128, 1], f32, tag="gsum")
            nc.vector.tensor_reduce(gsum[:], gap[:], AX.X, ALU.add)
            ppool = psb.tile([64, 1], f32, tag="psb")
            nc.tensor.matmul(ppool[:], ct["F2"][:], gsum[:])
            pooled = gp.tile([64, 1], f32, tag="pooled")
            nc.scalar.copy(pooled[:], ppool[:])
            plog = psb.tile([1, 4], f32, tag="psb")
            nc.tensor.matmul(plog[:], pooled[:], ct["WgT"][:])
            logit = gp.tile([1, 8], f32, tag="logit")
            nc.vector.memset(logit[:], 0.0)
            nc.vector.tensor_add(logit[:, 0:4], plog[:], ct["wgb"][:])
            nc.vector.tensor_reduce(logit[:, 4:5], logit[:, 0:4], AX.X, ALU.max)
            nc.vector.tensor_scalar(logit[:, 0:4], logit[:, 0:4],
                                    logit[:, 4:5], None, ALU.subtract)
            wrow = gp.tile([1, 4], f32, tag="wrow")
            nc.scalar.activation(wrow[:], logit[:, 0:4], AF.Exp,
                                 accum_out=logit[:, 5:6])
            nc.vector.reciprocal(logit[:, 6:7], logit[:, 5:6])
            nc.vector.tensor_scalar(wrow[:], wrow[:], logit[:, 6:7], None,
                                    ALU.mult)
            if DEBUG:
                nc.sync.dma_start(dbg["d_stats"][:], stats[:])
                nc.sync.dma_start(dbg["d_nstat"][:], nstat[:])
                nc.sync.dma_start(dbg["d_bias2"][:], bias2[:])
                nc.sync.dma_start(dbg["d_gap"][:], gap[:])
                nc.sync.dma_start(dbg["d_pooled"][:], pooled[:])
                nc.sync.dma_start(dbg["d_wrow"][:], wrow[:])
                nc.sync.dma_start(dbg["d_gs"][:], gs[:])
            ag_in = dr.tile([1, 4], f32)
            ag_out = dr.tile([8, 4], f32)
            nc.sync.dma_start(ag_in[:], wrow[:])
            nc.gpsimd.collective_compute(
                "AllGather", ALU.bypass, ins=[ag_in.opt()],
                outs=[ag_out.opt()],
                replica_groups=[list(range(NCORES))],
            )

            # ---------------- phase 1: DFT + squares (no weights) -----------
            Srs, Sis = [], []
            for b in range(B):
                fb = featb[b]
                uvb = uvp.tile([128, 2048], bf16, tag="uvb")
                p2s = []
                for cc in range(4):
                    p1 = ps1.tile([128, 512], f32, tag="p1")
                    for j in range(2):
                        c = 2 * cc + j
                        nc.tensor.matmul(p1[:, j * 256:(j + 1) * 256],
                                         fb[:, c * 128:(c + 1) * 128],
                                         ct["R1"][:])
                    if cc % 2 == 0:
                        nc.vector.tensor_copy(uvb[:, cc * 512:(cc + 1) * 512],
                                              p1[:])
                    else:
                        nc.scalar.copy(uvb[:, cc * 512:(cc + 1) * 512], p1[:])
                    p2 = ps2.tile([128, 260], f32, tag="p2")
                    for j in range(2):
                        base = cc * 512 + j * 256
                        nc.tensor.matmul(p2[:, j * 130:(j + 1) * 130],
                                         uvb[:, base:base + 128],
                                         ct["R2a"][:], start=True, stop=False)
                        nc.tensor.matmul(p2[:, j * 130:(j + 1) * 130],
                                         uvb[:, base + 128:base + 256],
                                         ct["R2b"][:], start=False, stop=True)
                    p2s.append(p2)
                X = xp.tile([128, 1040], bf16, tag="X")
                Xv = X[:].rearrange("p (r c x) -> p r c x", r=2, c=8)
                for cc in range(4):
                    src = p2s[cc][:].rearrange("p (j r x) -> p r j x", j=2, r=2)
                    dst = Xv[:, :, 2 * cc:2 * cc + 2, :]
                    if cc % 2 == 0:
                        nc.scalar.copy(dst, src)
                    else:
                        nc.vector.tensor_copy(dst, src)
                Xr, Xi = X[:, 0:520], X[:, 520:1040]
                t1 = tp.tile([128, 520], bf16, tag="t1")
                t2 = tp.tile([128, 520], bf16, tag="t2")
                Sr = sqp.tile([128, 520], bf16, tag=f"Sr{b}")
                Si = sqp.tile([128, 520], bf16, tag=f"Si{b}")
                nc.vector.tensor_add(t1[:], Xr, Xi)
                nc.vector.scalar_tensor_tensor(Si[:], Xr, 2.0, Xi,
                                               ALU.mult, ALU.mult)
                if b < 4:
                    nc.vector.tensor_sub(t2[:], Xr, Xi)
                    nc.vector.tensor_mul(Sr[:], t1[:], t2[:])
                else:
                    nc.gpsimd.tensor_sub(t2[:], Xr, Xi)
                    nc.gpsimd.tensor_mul(Sr[:], t1[:], t2[:])
                Srs.append(Sr)
                Sis.append(Si)
                if DEBUG and b == 0:
                    for nm, t in [("d_uv", uvb), ("d_X", X), ("d_Sr", Sr), ("d_Si", Si)]:
                        tf = gp.tile(list(t.shape), f32, tag="dump" + nm)
                        nc.vector.tensor_copy(tf[:], t[:])
                        nc.sync.dma_start(dbg[nm][:], tf[:])

            # ---------------- gathered weights -> per-sample masks ----------
            wT = gp.tile([4, 8], f32, tag="wT")
            nc.sync.dma_start(wT[:], ag_out[:].rearrange("b f -> f b"))
            pwcol = psb.tile([128, 8], f32, tag="psb")
            nc.tensor.matmul(pwcol[:], ct["E4"][:], wT[:])
            wcol = gp.tile([128, 8], f32, tag="wcol")
            nc.scalar.copy(wcol[:], pwcol[:])
            if DEBUG:
                nc.sync.dma_start(dbg["d_wcol"][:], wcol[:])
            wpat = []
            for b in range(B):
                t = gp.tile([128, 512], bf16, tag=f"wpat{b}")
                nc.vector.tensor_scalar(t[:], ct["maskJ"][:],
                                        wcol[:, b:b + 1], None, ALU.mult)
                wpat.append(t)

            # ---------------- phase 2: mix, product, inverse DFT ------------
            for b in range(B):
                fb = featb[b]
                Wm = wmp.tile([128, 1040], bf16, tag="Wm")
                Wmv = Wm[:].rearrange("p (r c x) -> p r c x", r=2, c=8)
                for cc in range(4):
                    pm = psm.tile([128, 260], f32, tag="psm")
                    for J in range(4):
                        nc.tensor.matmul(
                            pm[:], wpat[b][:, J * 128:(J + 1) * 128],
                            ftl[J][:, cc * 260:(cc + 1) * 260],
                            start=(J == 0), stop=(J == 3))
                    src = pm[:].rearrange("p (j r x) -> p r j x", j=2, r=2)
                    dst = Wmv[:, :, 2 * cc:2 * cc + 2, :]
                    if cc % 2 == 0:
                        nc.scalar.copy(dst, src)
                    else:
                        nc.vector.tensor_copy(dst, src)
                m1 = tp.tile([128, 520], bf16, tag="m1")
                m2 = tp.tile([128, 520], bf16, tag="m2")
                m3 = tp.tile([128, 520], bf16, tag="m3")
                m4 = tp.tile([128, 520], bf16, tag="m4")
                Dr = dp_.tile([128, 520], bf16, tag="Dr")
                Di = dp_.tile([128, 520], bf16, tag="Di")
                nc.vector.tensor_mul(m1[:], Srs[b][:], Wm[:, 0:520])
                nc.gpsimd.tensor_mul(m2[:], Sis[b][:], Wm[:, 520:1040])
                nc.vector.tensor_mul(m3[:], Srs[b][:], Wm[:, 520:1040])
                nc.gpsimd.tensor_mul(m4[:], Sis[b][:], Wm[:, 0:520])
                nc.vector.tensor_sub(Dr[:], m1[:], m2[:])
                nc.gpsimd.tensor_add(Di[:], m3[:], m4[:])
                if DEBUG and b == 0:
                    for nm, t in [("d_Wm", Wm), ("d_Dr", Dr), ("d_Di", Di)]:
                        tf = gp.tile(list(t.shape), f32, tag="dump" + nm)
                        nc.vector.tensor_copy(tf[:], t[:])
                        nc.sync.dma_start(dbg[nm][:], tf[:])
                z2b = z2p.tile([65, 2048], bf16, tag="z2b")
                for cc in range(4):
                    pA = psm.tile([65, 512], f32, tag="psm")
                    for j in range(2):
                        q = 2 * cc + j
                        nc.tensor.matmul(pA[:, j * 256:(j + 1) * 256],
                                         Dr[:, q * 65:(q + 1) * 65],
                                         ct["RA1"][:], start=True, stop=False)
                        nc.tensor.matmul(pA[:, j * 256:(j + 1) * 256],
                                         Di[:, q * 65:(q + 1) * 65],
                                         ct["RA2"][:], start=False, stop=True)
                    if cc % 2 == 0:
                        nc.vector.tensor_copy(z2b[:, cc * 512:(cc + 1) * 512],
                                              pA[:])
                    else:
                        nc.scalar.copy(z2b[:, cc * 512:(cc + 1) * 512], pA[:])
                if DEBUG and b == 0:
                    tf = gp.tile([65, 2048], f32, tag="dumpd_z2")
                    nc.vector.tensor_copy(tf[:], z2b[:])
                    nc.sync.dma_start(dbg["d_z2"][:], tf[:])
                for g in range(2):
                    pB = psb.tile([128, 512], f32, tag="psb")
                    for j in range(4):
                        q = 4 * g + j
                        nc.tensor.matmul(pB[:, j * 128:(j + 1) * 128],
                                         z2b[:, q * 256:q * 256 + 128],
                                         ct["RB1"][:], start=True, stop=False)
                        nc.tensor.matmul(pB[:, j * 128:(j + 1) * 128],
                                         z2b[:, q * 256 + 128:(q + 1) * 256],
                                         ct["RB2"][:], start=False, stop=True)
                    ot = op_.tile([128, 512], f32, tag="ot")
                    nc.vector.scalar_tensor_tensor(
                        ot[:], fb[:, g * 512:(g + 1) * 512],
                        ct["rcol"][:, 0:1], pB[:], ALU.mult, ALU.add)
                    nc.gpsimd.dma_start(
                        out_d[b, g * 4:(g + 1) * 4].rearrange("c h w -> h c w"),
                        ot[:].rearrange("p (c w) -> p c w", c=4))
    nc.compile()
    return nc


def _get_kernel():
    if "nc" not in _cache:
        _cache["nc"] = _build_kernel()
        _cache["consts"] = _build_constants()
    return _cache["nc"], _cache["consts"]


def kernel(**inputs):
    nc, consts = _get_kernel()
    Wblk, bprime, WgT, wgb, Wt, rw = _prep_params(inputs)
    feat = np.asarray(inputs["features"], np.float32)
    bf = ml_dtypes.bfloat16

    rcol = np.full((128, 1), rw, np.float32)
    base = {
        "R1": consts["R1"], "R2a": consts["R2a"], "R2b": consts["R2b"],
        "RA1": consts["RA1"], "RA2": consts["RA2"], "RB1": consts["RB1"],
        "RB2": consts["RB2"], "G16": consts["G16"], "E16": consts["E16"],
        "F2": consts["F2"], "E4": consts["E4"], "maskJ": consts["maskJ"],
        "Wblk": Wblk.astype(bf), "bprime": bprime, "WgT": WgT, "wgb": wgb,
        "rcol": rcol,
    }
    in_maps = []
    for k in range(NCORES):
        sl = slice(k * CS, (k + 1) * CS)
        # featf: [h, (b, c, w)]
        ff = feat[:, sl].transpose(2, 0, 1, 3).reshape(128, B * CS * W)
        # featg: [(t, c), (s)] with t = h-half
        fg = feat[k].reshape(C, 2, 64 * 128).transpose(1, 0, 2) \
                    .reshape(128, 64 * 128)
        # ftl: [J, (f, p), (c, ri, k2)]
        Wts = Wt[:, sl]                                   # [F, CS, H, WF]
        ftiles = np.empty((4, 128, CS * 2 * WF), np.float32)
        for J in range(4):
            blk = Wts[:, :, 32 * J:32 * J + 32, :]        # [F, CS, 32, WF]
            re = blk.real.astype(np.float32)
            im = blk.imag.astype(np.float32)
            stacked = np.stack([re, im], axis=3)          # [F, CS, 32, 2, WF]
            ftiles[J] = stacked.transpose(0, 2, 1, 3, 4).reshape(128, CS * 2 * WF)
        m = dict(base)
        m["featf"] = ff.astype(bf)
        m["featg"] = fg.astype(bf)
        m["ftl"] = ftiles.astype(bf)
        in_maps.append(m)

    res = run_bass_kernel_spmd(nc, in_maps, list(range(NCORES)))
    _cache["res"] = res
    out = np.empty((B, C, H, W), np.float32)
    for k in range(NCORES):
        out[:, k * CS:(k + 1) * CS] = res.results[k]["out"]
    return out
